# revision 1
# baseline (speedup 1.0000x reference)
"""BrainModel kernel for 8 TRN2 NeuronCores (raw bass, no Tile).

Reference computation:
    gathered = x[:, idx]                              # [B, O, C]
    pre = einsum('boc,oc->bo', gathered, w_sparse) + b_sparse
    new_x = sigmoid(pre)                              # [B, O]
    q = new_x[:, -N_MOTORS:] @ w_motor.T + b_motor    # [B, A]

Only the last N_MOTORS=256 rows of idx/w_sparse/b_sparse reach q, so the
other 98720 output neurons are dead code. We shard those 256 motor
neurons across the 8 cores (32 each).

Per-core device program (straight-line, 3 DMAs total):
  1. one HWDGE DMA loads a packed [128, 282] "aux" tile: the expanded
     block-sparse weights Wk (cols 0..255, directly usable as matmul
     lhsT), wmT (256..271), b_sparse (272), b_motor/8 (273), and the
     1024 gather indices as bitcast int32 (274..281).
  2. one SWDGE indirect DMA gathers 1024 rows of 64 floats (x[:, i] for
     each fan-in index i) from the transposed table xT=[N_NEURONS, B].
  3. 8 accumulating matmuls -> pre [32, B]; ScalarE sigmoid(+bias);
     matmul vs wmT -> q partial [A, B] (+ b_motor/8 on the PSUM->SBUF
     copy); one HWDGE DMA out.
Host sums the 8 partials and transposes to [B, A].

Raw bass keeps every instruction at <= 1 semaphore wait (the TRN2
walrus codegen rejects multi-wait Matmult/Drain encodings) and avoids
the Tile kernel-tail drain + all-engine barrier entirely.
"""

from contextlib import ExitStack

import numpy as np

import concourse.bass as bass
from concourse import mybir

N_NEURONS = 100000
N_MOTORS = 256
N_CONN = 32
N_ACT = 16
BATCH = 64
N_CORES = 8
M_PER_CORE = N_MOTORS // N_CORES  # 32 motor neurons per core
R = M_PER_CORE * N_CONN  # 1024 gathered x-rows per core
P = 128  # SBUF partitions
J = R // P  # 8 gather/matmul chunks

C_WMT = J * M_PER_CORE  # 256: wmT cols
C_BS = C_WMT + N_ACT  # 272: b_sparse col
C_BM = C_BS + 1  # 273: b_motor/8 col
C_IDX = C_BM + 1  # 274: idx cols (8 x int32 bitcast)
AUXC = C_IDX + J  # 282

_CACHE: dict = {}


def _build_nc() -> bass.Bass:
    f32 = mybir.dt.float32
    nc = bass.Bass(enable_partition_id=False)

    tbl = nc.declare_dram_parameter("tbl", [N_NEURONS, BATCH], f32, isOutput=False)
    aux = nc.declare_dram_parameter("aux", [P, AUXC], f32, isOutput=False)
    out = nc.declare_dram_parameter("out", [N_ACT, BATCH], f32, isOutput=True)

    with ExitStack() as ctx:
        aux_sb = ctx.enter_context(nc.sbuf_tensor("aux_sb", [P, AUXC], f32))
        G = ctx.enter_context(nc.sbuf_tensor("G", [P, J * BATCH], f32))
        s_sb = ctx.enter_context(nc.sbuf_tensor("s_sb", [M_PER_CORE, BATCH], f32))
        q_sb = ctx.enter_context(nc.sbuf_tensor("q_sb", [N_ACT, BATCH], f32))
        pre_ps = ctx.enter_context(nc.psum_tensor("pre_ps", [M_PER_CORE, BATCH], f32))
        q_ps = ctx.enter_context(nc.psum_tensor("q_ps", [N_ACT, BATCH], f32))
        isem = ctx.enter_context(nc.semaphore("isem"))
        wsem = ctx.enter_context(nc.semaphore("wsem"))
        odma_sem = ctx.enter_context(nc.semaphore("odma_sem"))
        # One completion sem per gather chunk: a single shared sem would be
        # racy -- each DMA's 16 increments come from 16 independent SDMA
        # engines, so a running count can reach 16*(j+1) before chunk j has
        # fully landed.
        gdma_sems = [
            ctx.enter_context(nc.semaphore(f"gdma_sem{j}")) for j in range(J)
        ]
        pe_sem = ctx.enter_context(nc.semaphore("pe_sem"))
        act_sem = ctx.enter_context(nc.semaphore("act_sem"))
        warm_sb = ctx.enter_context(nc.sbuf_tensor("warm_sb", [1, 1], f32))
        pad_sb = ctx.enter_context(nc.sbuf_tensor("pad_sb", [1, 1], f32))
        block = ctx.enter_context(nc.Block())

        @block.sync
        def _(sync):
            # idx columns first (small) so the gathers start ASAP; weights on
            # their own sem (completion order of two DMAs is not guaranteed).
            sync.dma_start(
                out=aux_sb[:, C_IDX:AUXC], in_=aux[:, C_IDX:AUXC]
            ).then_inc(isem, 16)
            sync.dma_start(out=aux_sb[:, :C_IDX], in_=aux[:, :C_IDX]).then_inc(
                wsem, 16
            )
            sync.wait_ge(odma_sem, 16)

        @block.gpsimd
        def _(gpsimd):
            gpsimd.wait_ge(isem, 16)
            # Cheap op right after the wait: the Pool sequencer has a ~1us
            # dispatch stall on the first instruction after a wait; let a
            # 1-element memset absorb it instead of the first gather.
            gpsimd.memset(pad_sb[:], 0)
            # The HW DGE consumes ONE index per partition per instruction:
            # partition p of the dest gets dest-free-size contiguous bytes
            # starting at tbl row idx[p]. So one gather per chunk j.
            for j in range(J):
                gpsimd.indirect_dma_start(
                    out=G[:, j * BATCH : (j + 1) * BATCH],
                    out_offset=None,
                    in_=tbl[:],
                    in_offset=bass.IndirectOffsetOnAxis(
                        ap=aux_sb[:, C_IDX + j : C_IDX + j + 1].bitcast(
                            mybir.dt.int32
                        ),
                        axis=0,
                    ),
                ).then_inc(gdma_sems[j], 16)

        @block.tensor
        def _(tensor):
            tensor.wait_ge(wsem, 16)
            # pre[m, b] = sum_{p,j} Wk[p, j*32+m] * x[b, idx_flat[p*J+j]]
            for j in range(J):
                tensor.wait_ge(gdma_sems[j], 16)
                mm = tensor.matmul(
                    pre_ps[:],
                    aux_sb[:, j * M_PER_CORE : (j + 1) * M_PER_CORE],
                    G[:, j * BATCH : (j + 1) * BATCH],
                    start=(j == 0),
                    stop=(j == J - 1),
                )
            mm.then_inc(pe_sem, 1)
            tensor.wait_ge(act_sem, 1)
            # q_part[a, b] = sum_m wmT[m, a] * s[m, b]
            tensor.matmul(
                q_ps[:],
                aux_sb[:M_PER_CORE, C_WMT : C_WMT + N_ACT],
                s_sb[:],
                start=True,
                stop=True,
            ).then_inc(pe_sem, 1)

        @block.scalar
        def _(scalar):
            # Dummy activation preloads the sigmoid LUT off the critical path
            # (the table load is ~1.3us and otherwise serializes after the
            # last matmul). Reads the already-landed idx region of aux_sb.
            scalar.wait_ge(isem, 16)
            scalar.activation(
                warm_sb[:],
                aux_sb[:1, C_IDX : C_IDX + 1],
                mybir.ActivationFunctionType.Sigmoid,
            )
            scalar.wait_ge(pe_sem, 1)
            # s = sigmoid(pre + b_sparse)
            scalar.activation(
                s_sb[:],
                pre_ps[:],
                mybir.ActivationFunctionType.Sigmoid,
                bias=aux_sb[:M_PER_CORE, C_BS : C_BS + 1],
            ).then_inc(act_sem, 1)
            scalar.wait_ge(pe_sem, 2)
            # q_sb = q_ps + b_motor/8 (PSUM -> SBUF)
            scalar.activation(
                q_sb[:],
                q_ps[:],
                mybir.ActivationFunctionType.Identity,
                bias=aux_sb[:N_ACT, C_BM : C_BM + 1],
            )
            # ScalarE is HWDGE-capable: issue the output DMA right here,
            # skipping a cross-engine semaphore hop to Sync.
            scalar.dma_start(out=out[:], in_=q_sb[:]).then_inc(odma_sem, 16)

    return nc


def _get_nc() -> bass.Bass:
    if "nc" not in _CACHE:
        _CACHE["nc"] = _build_nc()
    return _CACHE["nc"]


def make_in_maps(x, idx, w_sparse, b_sparse, w_motor, b_motor):
    """Shard FULL inputs into the 8 per-core input dicts."""
    x = np.asarray(x, dtype=np.float32)
    idx_m = np.asarray(idx)[-N_MOTORS:].astype(np.int32)  # [256, 32]
    w_m = np.asarray(w_sparse, dtype=np.float32)[-N_MOTORS:]  # [256, 32]
    b_m = np.asarray(b_sparse, dtype=np.float32)[-N_MOTORS:]  # [256]
    wm = np.asarray(w_motor, dtype=np.float32)  # [16, 256]
    bm = np.asarray(b_motor, dtype=np.float32)  # [16]

    xT = np.ascontiguousarray(x.T)  # [N_NEURONS, B] -- row i = x[:, i]

    r = np.arange(R)
    o_l, c = r // N_CONN, r % N_CONN
    p_r, j_r = r // J, r % J

    in_maps = []
    for k in range(N_CORES):
        rows = slice(k * M_PER_CORE, (k + 1) * M_PER_CORE)
        w_core = w_m[rows]  # [32, 32]

        aux = np.zeros((P, AUXC), np.float32)
        # Wk[p, j*32+m] = w[m, c] at r = p*J+j = m*32+c, else 0
        Wk = aux[:, :C_WMT]
        Wk[p_r, j_r * M_PER_CORE + o_l] = w_core[o_l, c]
        aux[:M_PER_CORE, C_WMT:C_BS] = wm[:, rows].T
        aux[:M_PER_CORE, C_BS] = b_m[rows]
        aux[:N_ACT, C_BM] = bm / N_CORES
        idx_tile = np.ascontiguousarray(idx_m[rows].reshape(P, J))  # int32
        aux[:, C_IDX:AUXC] = idx_tile.view(np.float32)

        in_maps.append({"tbl": xT, "aux": aux})
    return in_maps


def combine_outputs(partials):
    """Reduce the 8 per-core [A, B] partials to the full [B, A] output."""
    q = np.sum(np.stack(partials, axis=0), axis=0, dtype=np.float64)
    return np.ascontiguousarray(q.T).astype(np.float32)


def _ensure_trace_hook_importable():
    """bass_utils' axon trace path imports antenv.axon_hooks; some containers
    ship an antenv without it. Provide a null hook so trace degrades to a
    plain run instead of crashing."""
    import os

    if not os.environ.get("BASS_TRACE"):
        return
    try:
        import antenv.axon_hooks  # noqa: F401
    except ImportError:
        import sys
        import types

        import antenv

        m = types.ModuleType("antenv.axon_hooks")
        state = {"hook": None}
        m.set_axon_ntff_profile_hook = lambda h: state.__setitem__("hook", h)
        m.get_axon_ntff_profile_hook = lambda: state["hook"]
        sys.modules["antenv.axon_hooks"] = m
        antenv.axon_hooks = m


def kernel(x, idx, w_sparse, b_sparse, w_motor, b_motor):
    from concourse.bass_utils import run_bass_kernel_spmd

    _ensure_trace_hook_importable()
    nc = _get_nc()
    in_maps = make_in_maps(x, idx, w_sparse, b_sparse, w_motor, b_motor)
    res = run_bass_kernel_spmd(nc, in_maps, core_ids=list(range(N_CORES)))
    _CACHE["last_results"] = res
    return combine_outputs([res.results[k]["out"] for k in range(N_CORES)])

